# revision 8
# baseline (speedup 1.0000x reference)
"""Trainium2 Bass kernel: MultiHeadSelfAttention (B=2, S=2048, D=1024, H=16).

Self-contained. Accepts FULL inputs, returns FULL output.

Sharding (8 cores, SPMD, no collectives):
  core c -> batch b = c // 4, head-group g = c % 4 (4 heads = 256 dims).
  Each core computes q/k/v projections for its group, attention for its 4
  heads, and the row-parallel partial of the output projection
  (A_g @ Wo[:, g].T, shape (S, D)).  Host sums the 4 partials per batch.

Device-side math notes:
  - All matmuls run as float32r (full PE rate at N=512).
  - The reference masks ENTIRE query rows j >= valid_len to -1e6 before
    softmax, which makes those rows' attention exactly uniform (1/S each).
    We reproduce that exactly by multiplying q by the row mask: masked
    query -> scores all 0 -> exp all 1 -> uniform attention.
  - No max-subtraction in softmax: scores/8 are bounded (|s| < ~10), so
    exp() cannot overflow in fp32 and softmax is scale-invariant anyway.
  - Softmax denominator comes free from a ones-column appended to V
    (attn@V_aug yields sum(exp) in the extra output row).
  - bq/bk/bv are zeros in this problem's setup_inputs. bv/bo are folded in
    EXACTLY on the host (rows of attn sum to 1, so attn@(v+bv) = attn@v+bv).
    If bq/bk were ever nonzero we fall back to a numpy reference path.
"""

import numpy as np

B, S, D = 2, 2048, 1024
H, DH = 16, 64
HPG = 4                 # heads per group (per core)
GW = HPG * DH           # 256 group width
P = 128
N_CORES = 8

_PROG = None            # cached (nc, input_names)


def _emit(tc, aps):
    """Emit the per-core program into TileContext tc."""
    from contextlib import ExitStack

    import concourse.mybir as mybir

    nc = tc.nc
    f32 = mybir.dt.float32
    f32r = mybir.dt.float32r  # fp32 with 12-bit mantissa; full PE rate
    EXP = mybir.ActivationFunctionType.Exp

    xT, wqT, wkT, wvT, woT, mask, out = (
        aps["xT"], aps["wqT"], aps["wkT"], aps["wvT"], aps["woT"],
        aps["mask"], aps["out"],
    )


    ctx = ExitStack()
    with ctx:
        sb = ctx.enter_context(tc.tile_pool(name="sb", bufs=1))
        ps = ctx.enter_context(tc.tile_pool(name="ps", bufs=3, space="PSUM"))
        psav = ctx.enter_context(tc.tile_pool(name="psav", bufs=2, space="PSUM"))

        # persistent intermediates
        wo_sb = [sb.tile([P, D], f32r, name=f"wo{c}") for c in range(2)]
        for c in range(2):
            nc.sync.dma_start(wo_sb[c][:], woT[c * P:(c + 1) * P, :])
        q_sb = [sb.tile([P, S], f32r, name=f"q{p}") for p in range(2)]
        k_sb = [sb.tile([P, S], f32r, name=f"k{p}") for p in range(2)]
        v_sb = [sb.tile([P, HPG, DH + 1], f32r, name=f"v{t}") for t in range(16)]
        a_sb = [sb.tile([P, S], f32r, name=f"a{c}") for c in range(2)]

        # ---- phase 1: projections (X/W/mask pool scoped to this phase) --
        with tc.tile_pool(name="xw", bufs=1) as xw:
            x_sb = [xw.tile([P, S], f32r, name=f"x{d}") for d in range(8)]
            for d in range(8):
                nc.sync.dma_start(x_sb[d][:], xT[d * P:(d + 1) * P, :])
            wq_sb = [xw.tile([P, GW], f32r, name=f"wq{d}") for d in range(8)]
            wk_sb = [xw.tile([P, GW], f32r, name=f"wk{d}") for d in range(8)]
            wv_sb = [xw.tile([P, GW], f32r, name=f"wv{d}") for d in range(8)]
            for d in range(8):
                nc.sync.dma_start(wq_sb[d][:], wqT[d * P:(d + 1) * P, :])
                nc.sync.dma_start(wk_sb[d][:], wkT[d * P:(d + 1) * P, :])
                nc.sync.dma_start(wv_sb[d][:], wvT[d * P:(d + 1) * P, :])
            mk_sb = [xw.tile([P, S], f32, name=f"mk{p}") for p in range(2)]
            for p in range(2):
                nc.sync.dma_start(mk_sb[p][:], mask[p])

            # q/k projections: out = W_g @ X.T in (head-dim, token) layout
            for wt, dst, is_q in ((wq_sb, q_sb, True), (wk_sb, k_sb, False)):
                for mt in range(2):
                    for n4 in range(4):
                        pt = ps.tile([P, 1024], f32, name="ps")[:, :512]
                        for d in range(8):
                            nc.tensor.matmul(
                                pt,
                                (wt[d][:, mt * P:(mt + 1) * P]),
                                (x_sb[d][:, n4 * 512:(n4 + 1) * 512]),
                                start=(d == 0), stop=(d == 7),
                            )
                        dslice = dst[mt][:, n4 * 512:(n4 + 1) * 512]
                        if is_q:
                            # fold the row mask into q (masked query -> q = 0)
                            nc.vector.tensor_mul(
                                dslice, pt,
                                mk_sb[mt][:, n4 * 512:(n4 + 1) * 512])
                        else:
                            nc.vector.tensor_copy(dslice, pt)

            # v projection in (token, head-dim) layout + ones column
            for t in range(16):
                pt = ps.tile([P, 1024], f32, name="ps")[:, :GW]
                for d in range(8):
                    nc.tensor.matmul(
                        pt,
                        (x_sb[d][:, t * P:(t + 1) * P]),
                        (wv_sb[d][:]),
                        start=(d == 0), stop=(d == 7),
                    )
                # ones column at [:, :, DH]; memset lacks an f32r value
                # type, so write through an f32 view (same bits for 1.0)
                nc.any.memset(v_sb[t][:].bitcast(f32), 1.0)
                nc.vector.tensor_copy(
                    v_sb[t][:, :, 0:DH],
                    pt.rearrange("p (h e) -> p h e", h=HPG),
                )

        # ---- phase 2 pools ----------------------------------------------
        rot = ctx.enter_context(tc.tile_pool(name="rot", bufs=4))
        ost = ctx.enter_context(tc.tile_pool(name="ost", bufs=3))
        sml = ctx.enter_context(tc.tile_pool(name="sml", bufs=3))
        scr = ctx.enter_context(tc.tile_pool(name="scr", bufs=3, space="DRAM"))

        # ---- attention + output projection, per 512-query chunk ---------
        for i4 in range(4):
            qs = slice(i4 * 512, (i4 + 1) * 512)
            for pair in range(2):
                pav = [psav.tile([DH + 1, 512], f32, name="psav")
                       for _ in range(2)]
                for jtp in range(8):
                    pse = [ps.tile([P, 1024], f32, name="ps")
                           for _ in range(2)]
                    for jj in range(2):
                        jt = jtp * 2 + jj
                        for rr in range(2):
                            rows = slice(64 * rr, 64 * rr + 64)
                            # scores^T = k @ q^T for head (2*pair + rr)
                            nc.tensor.matmul(
                                pse[rr][:, jj * 512:(jj + 1) * 512],
                                (k_sb[pair][rows, jt * P:(jt + 1) * P]),
                                (q_sb[pair][rows, qs]),
                                start=True, stop=True,
                            )
                    ex = [rot.tile([P, 1024], f32r, name="ex")
                          for _ in range(2)]
                    for rr in range(2):
                        nc.scalar.activation(
                            ex[rr][:], pse[rr][:], EXP, bias=0.0, scale=0.125)
                    for rr in range(2):
                        h = 2 * pair + rr
                        for jj in range(2):
                            jt = jtp * 2 + jj
                            nc.tensor.matmul(
                                pav[rr][:],
                                (v_sb[jt][:, h, :]),
                                (ex[rr][:, jj * 512:(jj + 1) * 512]),
                                start=(jtp == 0 and jj == 0),
                                stop=(jtp == 7 and jj == 1),
                            )
                for rr in range(2):
                    rc = sml.tile([1, 512], f32, name="rc")
                    # (reciprocal_approx_fast mis-executes on this
                    # toolchain's HW path; plain reciprocal is correct)
                    nc.vector.reciprocal(rc[:], pav[rr][DH:DH + 1, :])
                    # partition-broadcast via a DRAM bounce (DMA cannot read
                    # SBUF with partition step 0, but DRAM sources can)
                    sc = scr.tile([1, 512], f32, name="sc")
                    nc.sync.dma_start(sc[:], rc[:])
                    rcb = sml.tile([64, 512], f32, name="rcb")
                    nc.sync.dma_start(rcb[:], sc[:].partition_broadcast(64))
                    nc.vector.tensor_mul(
                        a_sb[pair][64 * rr:64 * rr + 64, qs],
                        pav[rr][0:DH, :],
                        rcb[:],
                    )
            # output projection for this query chunk
            for t4 in range(4):
                t = i4 * 4 + t4
                ot = ost.tile([P, D], f32, name="ot")
                for n2 in range(2):
                    pf = ps.tile([P, 1024], f32, name="ps")[:, :512]
                    for c in range(2):
                        nc.tensor.matmul(
                            pf,
                            (a_sb[c][:, t * P:(t + 1) * P]),
                            (wo_sb[c][:, n2 * 512:(n2 + 1) * 512]),
                            start=(c == 0), stop=(c == 1),
                        )
                    nc.vector.tensor_copy(ot[:, n2 * 512:(n2 + 1) * 512], pf)
                nc.sync.dma_start(out[t * P:(t + 1) * P, :], ot[:])


def build_program():
    """Build + schedule + compile the per-core Bass program (cached)."""
    global _PROG
    if _PROG is not None:
        return _PROG

    import concourse.mybir as mybir
    import concourse.tile as tile
    from concourse import bacc

    nc = bacc.Bacc("TRN2", target_bir_lowering=False, debug=False)
    f32 = mybir.dt.float32
    f32r = mybir.dt.float32r
    aps = {
        "xT": nc.dram_tensor("xT", [D, S], f32r, kind="ExternalInput").ap(),
        "wqT": nc.dram_tensor("wqT", [D, GW], f32r, kind="ExternalInput").ap(),
        "wkT": nc.dram_tensor("wkT", [D, GW], f32r, kind="ExternalInput").ap(),
        "wvT": nc.dram_tensor("wvT", [D, GW], f32r, kind="ExternalInput").ap(),
        "woT": nc.dram_tensor("woT", [GW, D], f32r, kind="ExternalInput").ap(),
        "mask": nc.dram_tensor("mask", [2, P, S], f32,
                               kind="ExternalInput").ap(),
        "out": nc.dram_tensor("out", [S, D], f32, kind="ExternalOutput").ap(),
    }
    with tile.TileContext(nc) as tc:
        _emit(tc, aps)
    nc.compile()
    _PROG = nc
    return nc


def _to_f32r(a):
    """Round fp32 -> fp32r (12-bit mantissa, round-half-to-even).

    Bit-exact with neuronxcc's fp32_to_fp32r; the PE requires fp32r matmul
    operands to be pre-rounded.
    """
    a = np.ascontiguousarray(a, dtype=np.float32)
    bits = a.view(np.uint32).astype(np.uint64)
    r = (bits + 0x7FF + ((bits >> 12) & 1)) & 0xFFFFF000
    return r.astype(np.uint32).view(np.float32).reshape(a.shape)


def make_in_maps(X, Wq, Wk, Wv, Wo, valid_lens):
    """Host-side sharding: build the 8 per-core input maps."""
    X = np.asarray(X, dtype=np.float32)
    valid = np.asarray(valid_lens).reshape(B, H)
    iota = np.arange(S)
    in_maps = []
    xTs = [_to_f32r(X[b].T) for b in range(B)]
    for c in range(N_CORES):
        b, g = divmod(c, HPG)
        cols = slice(g * GW, (g + 1) * GW)
        mask = np.empty((2, P, S), dtype=np.float32)
        for p in range(2):
            for rr in range(2):
                h = HPG * g + 2 * p + rr
                mask[p, 64 * rr:64 * rr + 64, :] = (
                    iota < int(valid[b, h])).astype(np.float32)[None, :]
        in_maps.append({
            "xT": xTs[b],
            "wqT": _to_f32r(np.asarray(Wq)[cols, :].T),
            "wkT": _to_f32r(np.asarray(Wk)[cols, :].T),
            "wvT": _to_f32r(np.asarray(Wv)[cols, :].T),
            "woT": _to_f32r(np.asarray(Wo)[:, cols].T),
            "mask": mask,
        })
    return in_maps


def assemble(results, Wo, bv, bo):
    """Host-side unshard: sum row-parallel partials, fold bv/bo exactly."""
    out = np.zeros((B, S, D), dtype=np.float32)
    for c in range(N_CORES):
        b = c // HPG
        out[b] += results[c]["out"]
    bias = (np.asarray(bv, np.float32) @ np.asarray(Wo, np.float32).T
            + np.asarray(bo, np.float32))
    out += bias[None, None, :]
    return out


def _numpy_fallback(X, Wq, bq, Wk, bk, Wv, bv, Wo, bo, valid_lens):
    X = np.asarray(X, np.float32)
    q = (X @ np.asarray(Wq, np.float32).T + np.asarray(bq, np.float32))
    k = (X @ np.asarray(Wk, np.float32).T + np.asarray(bk, np.float32))
    v = (X @ np.asarray(Wv, np.float32).T + np.asarray(bv, np.float32))

    def split(y):
        return (y.reshape(B, S, H, DH).transpose(0, 2, 1, 3)
                .reshape(B * H, S, DH))

    q, k, v = split(q), split(k), split(v)
    s = np.einsum("bqd,bkd->bqk", q, k) / np.sqrt(DH).astype(np.float32)
    rm = (np.arange(S)[None, :]
          < np.asarray(valid_lens).reshape(-1)[:, None])
    s = np.where(rm[:, :, None], s, -1e6)
    s = s - s.max(axis=-1, keepdims=True)
    e = np.exp(s)
    attn = e / e.sum(axis=-1, keepdims=True)
    o = np.einsum("bqk,bkd->bqd", attn, v)
    o = o.reshape(B, H, S, DH).transpose(0, 2, 1, 3).reshape(B, S, D)
    return o @ np.asarray(Wo, np.float32).T + np.asarray(bo, np.float32)


def run_cores(in_maps, trace=False, **kw):
    """Run the compiled program on cores 0-7."""
    from concourse.bass_utils import run_bass_kernel_spmd

    nc = build_program()
    return run_bass_kernel_spmd(nc, in_maps, list(range(N_CORES)),
                                trace=trace, **kw)


def kernel(X, Wq, bq, Wk, bk, Wv, bv, Wo, bo, valid_lens):
    if np.any(np.asarray(bq)) or np.any(np.asarray(bk)):
        # never the case for this problem's setup_inputs (zeros);
        # exact fallback kept for safety.
        return _numpy_fallback(X, Wq, bq, Wk, bk, Wv, bv, Wo, bo, valid_lens)
    in_maps = make_in_maps(X, Wq, Wk, Wv, Wo, valid_lens)
    res = run_cores(in_maps, trace=False)
    return assemble(res.results, Wo, bv, bo)


# revision 9
# speedup vs baseline: 1.0786x; 1.0786x over previous
"""Trainium2 Bass kernel: MultiHeadSelfAttention (B=2, S=2048, D=1024, H=16).

Self-contained. Accepts FULL inputs, returns FULL output.

Sharding (8 cores, SPMD, no collectives):
  core c -> batch b = c // 4, head-group g = c % 4 (4 heads = 256 dims).
  Each core computes q/k/v projections for its group, attention for its 4
  heads, and the row-parallel partial of the output projection
  (A_g @ Wo[:, g].T, shape (S, D)).  Host sums the 4 partials per batch.

Device-side math notes:
  - All matmuls run in bf16 (fp32 PSUM accumulation).
  - The reference masks ENTIRE query rows j >= valid_len to -1e6 before
    softmax, which makes those rows' attention exactly uniform (1/S each).
    We reproduce that exactly by multiplying q by the row mask: masked
    query -> scores all 0 -> exp all 1 -> uniform attention.
  - No max-subtraction in softmax: scores/8 are bounded (|s| < ~10), so
    exp() cannot overflow in fp32 and softmax is scale-invariant anyway.
  - Softmax denominator comes free from a ones-column appended to V
    (attn@V_aug yields sum(exp) in the extra output row).
  - bq/bk/bv are zeros in this problem's setup_inputs. bv/bo are folded in
    EXACTLY on the host (rows of attn sum to 1, so attn@(v+bv) = attn@v+bv).
    If bq/bk were ever nonzero we fall back to a numpy reference path.
"""

import numpy as np

B, S, D = 2, 2048, 1024
H, DH = 16, 64
HPG = 4                 # heads per group (per core)
GW = HPG * DH           # 256 group width
P = 128
N_CORES = 8

_PROG = None            # cached (nc, input_names)


def _emit(tc, aps):
    """Emit the per-core program into TileContext tc."""
    from contextlib import ExitStack

    import concourse.mybir as mybir

    nc = tc.nc
    f32 = mybir.dt.float32
    bf16 = mybir.dt.bfloat16
    EXP = mybir.ActivationFunctionType.Exp

    xT, wqT, wkT, wvT, woT, mask, out = (
        aps["xT"], aps["wqT"], aps["wkT"], aps["wvT"], aps["woT"],
        aps["mask"], aps["out"],
    )


    ctx = ExitStack()
    with ctx:
        sb = ctx.enter_context(tc.tile_pool(name="sb", bufs=1))
        ps = ctx.enter_context(tc.tile_pool(name="ps", bufs=3, space="PSUM"))
        psav = ctx.enter_context(tc.tile_pool(name="psav", bufs=2, space="PSUM"))

        # persistent intermediates
        wo_sb = [sb.tile([P, D], bf16, name=f"wo{c}") for c in range(2)]
        for c in range(2):
            nc.sync.dma_start(wo_sb[c][:], woT[c * P:(c + 1) * P, :])
        q_sb = [sb.tile([P, S], bf16, name=f"q{p}") for p in range(2)]
        k_sb = [sb.tile([P, S], bf16, name=f"k{p}") for p in range(2)]
        v_sb = [sb.tile([P, HPG, DH + 1], bf16, name=f"v{t}") for t in range(16)]
        a_sb = [sb.tile([P, S], bf16, name=f"a{c}") for c in range(2)]

        # ---- phase 1: projections (X/W/mask pool scoped to this phase) --
        with tc.tile_pool(name="xw", bufs=1) as xw:
            x_sb = [xw.tile([P, S], bf16, name=f"x{d}") for d in range(8)]
            for d in range(8):
                nc.sync.dma_start(x_sb[d][:], xT[d * P:(d + 1) * P, :])
            wq_sb = [xw.tile([P, GW], bf16, name=f"wq{d}") for d in range(8)]
            wk_sb = [xw.tile([P, GW], bf16, name=f"wk{d}") for d in range(8)]
            wv_sb = [xw.tile([P, GW], bf16, name=f"wv{d}") for d in range(8)]
            for d in range(8):
                nc.sync.dma_start(wq_sb[d][:], wqT[d * P:(d + 1) * P, :])
                nc.sync.dma_start(wk_sb[d][:], wkT[d * P:(d + 1) * P, :])
                nc.sync.dma_start(wv_sb[d][:], wvT[d * P:(d + 1) * P, :])
            mk_sb = [xw.tile([P, S], f32, name=f"mk{p}") for p in range(2)]
            for p in range(2):
                nc.sync.dma_start(mk_sb[p][:], mask[p])

            # q/k projections: out = W_g @ X.T in (head-dim, token) layout
            for wt, dst, is_q in ((wq_sb, q_sb, True), (wk_sb, k_sb, False)):
                for mt in range(2):
                    for n4 in range(4):
                        pt = ps.tile([P, 1024], f32, name="ps")[:, :512]
                        for d in range(8):
                            nc.tensor.matmul(
                                pt,
                                (wt[d][:, mt * P:(mt + 1) * P]),
                                (x_sb[d][:, n4 * 512:(n4 + 1) * 512]),
                                start=(d == 0), stop=(d == 7),
                            )
                        dslice = dst[mt][:, n4 * 512:(n4 + 1) * 512]
                        if is_q:
                            # fold the row mask into q (masked query -> q = 0)
                            nc.vector.tensor_mul(
                                dslice, pt,
                                mk_sb[mt][:, n4 * 512:(n4 + 1) * 512])
                        else:
                            nc.vector.tensor_copy(dslice, pt)

            # v projection in (token, head-dim) layout + ones column
            for t in range(16):
                pt = ps.tile([P, 1024], f32, name="ps")[:, :GW]
                for d in range(8):
                    nc.tensor.matmul(
                        pt,
                        (x_sb[d][:, t * P:(t + 1) * P]),
                        (wv_sb[d][:]),
                        start=(d == 0), stop=(d == 7),
                    )
                nc.any.memset(v_sb[t][:], 1.0)  # ones column at [:, :, DH]
                nc.vector.tensor_copy(
                    v_sb[t][:, :, 0:DH],
                    pt.rearrange("p (h e) -> p h e", h=HPG),
                )

        # ---- phase 2 pools ----------------------------------------------
        rot = ctx.enter_context(tc.tile_pool(name="rot", bufs=4))
        ost = ctx.enter_context(tc.tile_pool(name="ost", bufs=3))
        sml = ctx.enter_context(tc.tile_pool(name="sml", bufs=3))
        scr = ctx.enter_context(tc.tile_pool(name="scr", bufs=3, space="DRAM"))

        # ---- attention + output projection, per 512-query chunk ---------
        for i4 in range(4):
            qs = slice(i4 * 512, (i4 + 1) * 512)
            for pair in range(2):
                pav = [psav.tile([DH + 1, 512], f32, name="psav")
                       for _ in range(2)]
                for jtp in range(8):
                    pse = [ps.tile([P, 1024], f32, name="ps")
                           for _ in range(2)]
                    for jj in range(2):
                        jt = jtp * 2 + jj
                        for rr in range(2):
                            rows = slice(64 * rr, 64 * rr + 64)
                            # scores^T = k @ q^T for head (2*pair + rr)
                            nc.tensor.matmul(
                                pse[rr][:, jj * 512:(jj + 1) * 512],
                                (k_sb[pair][rows, jt * P:(jt + 1) * P]),
                                (q_sb[pair][rows, qs]),
                                start=True, stop=True,
                            )
                    ex = [rot.tile([P, 1024], bf16, name="ex")
                          for _ in range(2)]
                    for rr in range(2):
                        nc.scalar.activation(
                            ex[rr][:], pse[rr][:], EXP, bias=0.0, scale=0.125)
                    for rr in range(2):
                        h = 2 * pair + rr
                        for jj in range(2):
                            jt = jtp * 2 + jj
                            nc.tensor.matmul(
                                pav[rr][:],
                                (v_sb[jt][:, h, :]),
                                (ex[rr][:, jj * 512:(jj + 1) * 512]),
                                start=(jtp == 0 and jj == 0),
                                stop=(jtp == 7 and jj == 1),
                            )
                for rr in range(2):
                    rc = sml.tile([1, 512], f32, name="rc")
                    # (reciprocal_approx_fast mis-executes on this
                    # toolchain's HW path; plain reciprocal is correct)
                    nc.vector.reciprocal(rc[:], pav[rr][DH:DH + 1, :])
                    # partition-broadcast via a DRAM bounce (DMA cannot read
                    # SBUF with partition step 0, but DRAM sources can)
                    sc = scr.tile([1, 512], f32, name="sc")
                    nc.sync.dma_start(sc[:], rc[:])
                    rcb = sml.tile([64, 512], f32, name="rcb")
                    nc.sync.dma_start(rcb[:], sc[:].partition_broadcast(64))
                    nc.vector.tensor_mul(
                        a_sb[pair][64 * rr:64 * rr + 64, qs],
                        pav[rr][0:DH, :],
                        rcb[:],
                    )
            # output projection for this query chunk
            for t4 in range(4):
                t = i4 * 4 + t4
                ot = ost.tile([P, D], f32, name="ot")
                for n2 in range(2):
                    pf = ps.tile([P, 1024], f32, name="ps")[:, :512]
                    for c in range(2):
                        nc.tensor.matmul(
                            pf,
                            (a_sb[c][:, t * P:(t + 1) * P]),
                            (wo_sb[c][:, n2 * 512:(n2 + 1) * 512]),
                            start=(c == 0), stop=(c == 1),
                        )
                    nc.vector.tensor_copy(ot[:, n2 * 512:(n2 + 1) * 512], pf)
                nc.sync.dma_start(out[t * P:(t + 1) * P, :], ot[:])


def build_program():
    """Build + schedule + compile the per-core Bass program (cached)."""
    global _PROG
    if _PROG is not None:
        return _PROG

    import concourse.mybir as mybir
    import concourse.tile as tile
    from concourse import bacc

    nc = bacc.Bacc("TRN2", target_bir_lowering=False, debug=False)
    f32 = mybir.dt.float32
    bf16 = mybir.dt.bfloat16
    aps = {
        "xT": nc.dram_tensor("xT", [D, S], bf16, kind="ExternalInput").ap(),
        "wqT": nc.dram_tensor("wqT", [D, GW], bf16, kind="ExternalInput").ap(),
        "wkT": nc.dram_tensor("wkT", [D, GW], bf16, kind="ExternalInput").ap(),
        "wvT": nc.dram_tensor("wvT", [D, GW], bf16, kind="ExternalInput").ap(),
        "woT": nc.dram_tensor("woT", [GW, D], bf16, kind="ExternalInput").ap(),
        "mask": nc.dram_tensor("mask", [2, P, S], f32,
                               kind="ExternalInput").ap(),
        "out": nc.dram_tensor("out", [S, D], f32, kind="ExternalOutput").ap(),
    }
    with tile.TileContext(nc) as tc:
        _emit(tc, aps)
    nc.compile()
    _PROG = nc
    return nc


def _to_bf16(a):
    import ml_dtypes
    return np.ascontiguousarray(np.asarray(a, dtype=np.float32)
                                .astype(ml_dtypes.bfloat16))


def make_in_maps(X, Wq, Wk, Wv, Wo, valid_lens):
    """Host-side sharding: build the 8 per-core input maps."""
    X = np.asarray(X, dtype=np.float32)
    valid = np.asarray(valid_lens).reshape(B, H)
    iota = np.arange(S)
    in_maps = []
    xTs = [_to_bf16(X[b].T) for b in range(B)]
    for c in range(N_CORES):
        b, g = divmod(c, HPG)
        cols = slice(g * GW, (g + 1) * GW)
        mask = np.empty((2, P, S), dtype=np.float32)
        for p in range(2):
            for rr in range(2):
                h = HPG * g + 2 * p + rr
                mask[p, 64 * rr:64 * rr + 64, :] = (
                    iota < int(valid[b, h])).astype(np.float32)[None, :]
        in_maps.append({
            "xT": xTs[b],
            "wqT": _to_bf16(np.asarray(Wq)[cols, :].T),
            "wkT": _to_bf16(np.asarray(Wk)[cols, :].T),
            "wvT": _to_bf16(np.asarray(Wv)[cols, :].T),
            "woT": _to_bf16(np.asarray(Wo)[:, cols].T),
            "mask": mask,
        })
    return in_maps


def assemble(results, Wo, bv, bo):
    """Host-side unshard: sum row-parallel partials, fold bv/bo exactly."""
    out = np.zeros((B, S, D), dtype=np.float32)
    for c in range(N_CORES):
        b = c // HPG
        out[b] += results[c]["out"]
    bias = (np.asarray(bv, np.float32) @ np.asarray(Wo, np.float32).T
            + np.asarray(bo, np.float32))
    out += bias[None, None, :]
    return out


def _numpy_fallback(X, Wq, bq, Wk, bk, Wv, bv, Wo, bo, valid_lens):
    X = np.asarray(X, np.float32)
    q = (X @ np.asarray(Wq, np.float32).T + np.asarray(bq, np.float32))
    k = (X @ np.asarray(Wk, np.float32).T + np.asarray(bk, np.float32))
    v = (X @ np.asarray(Wv, np.float32).T + np.asarray(bv, np.float32))

    def split(y):
        return (y.reshape(B, S, H, DH).transpose(0, 2, 1, 3)
                .reshape(B * H, S, DH))

    q, k, v = split(q), split(k), split(v)
    s = np.einsum("bqd,bkd->bqk", q, k) / np.sqrt(DH).astype(np.float32)
    rm = (np.arange(S)[None, :]
          < np.asarray(valid_lens).reshape(-1)[:, None])
    s = np.where(rm[:, :, None], s, -1e6)
    s = s - s.max(axis=-1, keepdims=True)
    e = np.exp(s)
    attn = e / e.sum(axis=-1, keepdims=True)
    o = np.einsum("bqk,bkd->bqd", attn, v)
    o = o.reshape(B, H, S, DH).transpose(0, 2, 1, 3).reshape(B, S, D)
    return o @ np.asarray(Wo, np.float32).T + np.asarray(bo, np.float32)


def run_cores(in_maps, trace=False, **kw):
    """Run the compiled program on cores 0-7."""
    from concourse.bass_utils import run_bass_kernel_spmd

    nc = build_program()
    return run_bass_kernel_spmd(nc, in_maps, list(range(N_CORES)),
                                trace=trace, **kw)


def kernel(X, Wq, bq, Wk, bk, Wv, bv, Wo, bo, valid_lens):
    if np.any(np.asarray(bq)) or np.any(np.asarray(bk)):
        # never the case for this problem's setup_inputs (zeros);
        # exact fallback kept for safety.
        return _numpy_fallback(X, Wq, bq, Wk, bk, Wv, bv, Wo, bo, valid_lens)
    in_maps = make_in_maps(X, Wq, Wk, Wv, Wo, valid_lens)
    res = run_cores(in_maps, trace=False)
    return assemble(res.results, Wo, bv, bo)


# revision 10
# speedup vs baseline: 1.2173x; 1.1286x over previous
"""Trainium2 Bass kernel: MultiHeadSelfAttention (B=2, S=2048, D=1024, H=16).

Self-contained. Accepts FULL inputs, returns FULL output.

Sharding (8 cores, SPMD, no collectives):
  core c -> batch b = c // 4, lane j = c % 4. Within a batch the 16 heads
  are sorted by valid_len (desc) and dealt round-robin to the 4 lanes, so
  slot i on every core holds a head from rank-quartet i. Each core computes
  q/k/v projections for its 4 heads, attention, and the row-parallel
  partial of the output projection (A @ Wo[:, heads].T, shape (S, D)).
  Host sums the 4 partials per batch.

The program is specialized to per-slot QUERY BUDGETS: budget[i] =
ceil(max valid_len in rank-quartet i / 512) * 512. Query chunks beyond a
slot's budget are entirely masked rows, whose attention output is exactly
uniform (= mean of V), so they are filled from a precomputed mean-V column
instead of being computed. One program serves all 8 cores; distinct
budget tuples (at most 35) compile separately and are cached.

Device-side math notes:
  - All matmuls run in bf16 (fp32 PSUM accumulation).
  - The reference masks ENTIRE query rows j >= valid_len to -1e6 before
    softmax, making those rows' attention exactly uniform (1/S each). For
    masked rows inside a computed chunk we multiply q by the row mask:
    masked query -> scores all 0 -> exp all 1 -> uniform attention.
  - No max-subtraction in softmax: scores/8 are bounded (|s| < ~10), so
    exp() cannot overflow in fp32 and softmax is scale-invariant anyway.
  - Softmax denominator comes free from a ones-column appended to V
    (attn@V_aug yields sum(exp) in the extra output row).
  - bq/bk/bv are zeros in this problem's setup_inputs. bv/bo are folded in
    EXACTLY on the host (rows of attn sum to 1, so attn@(v+bv) = attn@v+bv).
    If bq/bk were ever nonzero we fall back to a numpy reference path.
"""

import numpy as np

B, S, D = 2, 2048, 1024
H, DH = 16, 64
HPG = 4                 # heads per core
GW = HPG * DH           # 256
P = 128
N_CORES = 8
NCH = S // 512          # query chunks

_PROGS = {}             # budgets tuple -> compiled Bacc


def _to_bf16(a):
    import ml_dtypes
    return np.ascontiguousarray(np.asarray(a, dtype=np.float32)
                                .astype(ml_dtypes.bfloat16))


def _emit(tc, aps, budgets):
    """Emit the per-core program. budgets: 4 per-slot query budgets."""
    from contextlib import ExitStack

    import concourse.mybir as mybir

    nc = tc.nc
    f32 = mybir.dt.float32
    bf16 = mybir.dt.bfloat16
    EXP = mybir.ActivationFunctionType.Exp
    COPY = mybir.ActivationFunctionType.Copy

    xT, wqT, wkT, wvT, woT, mask, out = (
        aps["xT"], aps["wqT"], aps["wkT"], aps["wvT"], aps["woT"],
        aps["mask"], aps["out"],
    )
    nchunks = [b // 512 for b in budgets]     # computed chunks per slot
    qmax = max(budgets)

    ctx = ExitStack()
    with ctx:
        sb = ctx.enter_context(tc.tile_pool(name="sb", bufs=1))
        ps = ctx.enter_context(tc.tile_pool(name="ps", bufs=3, space="PSUM"))
        psav = ctx.enter_context(tc.tile_pool(name="psav", bufs=2, space="PSUM"))

        # persistent intermediates
        wo_sb = [sb.tile([P, D], bf16, name=f"wo{c}") for c in range(2)]
        for c in range(2):
            nc.sync.dma_start(wo_sb[c][:], woT[c * P:(c + 1) * P, :])
        q_sb = [sb.tile([P, S], bf16, name=f"q{p}") for p in range(2)]
        k_sb = [sb.tile([P, S], bf16, name=f"k{p}") for p in range(2)]
        v_sb = [sb.tile([P, HPG, DH + 1], bf16, name=f"v{t}") for t in range(16)]
        a_sb = [sb.tile([P, S], bf16, name=f"a{c}") for c in range(2)]
        meanv = sb.tile([64, HPG], bf16, name="meanv")

        # ---- phase 1: projections (X/W/mask pool scoped to this phase) --
        with tc.tile_pool(name="xw", bufs=1) as xw:
            x_sb = [xw.tile([P, S], bf16, name=f"x{d}") for d in range(8)]
            for d in range(8):
                nc.sync.dma_start(x_sb[d][:], xT[d * P:(d + 1) * P, :])
            wq_sb = [xw.tile([P, GW], bf16, name=f"wq{d}") for d in range(8)]
            wk_sb = [xw.tile([P, GW], bf16, name=f"wk{d}") for d in range(8)]
            wv_sb = [xw.tile([P, GW], bf16, name=f"wv{d}") for d in range(8)]
            for d in range(8):
                nc.sync.dma_start(wq_sb[d][:], wqT[d * P:(d + 1) * P, :])
                nc.sync.dma_start(wk_sb[d][:], wkT[d * P:(d + 1) * P, :])
                nc.sync.dma_start(wv_sb[d][:], wvT[d * P:(d + 1) * P, :])
            mk_sb = [xw.tile([P, S], f32, name=f"mk{p}") for p in range(2)]
            for p in range(2):
                nc.sync.dma_start(mk_sb[p][:], mask[p])

            # q/k projections in (head-dim, token) layout; q-chunks beyond
            # the pair's budget are never read, so skip those matmuls
            for wt, dst, is_q in ((wq_sb, q_sb, True), (wk_sb, k_sb, False)):
                for mt in range(2):
                    pair_budget = max(budgets[2 * mt], budgets[2 * mt + 1])
                    for n4 in range(NCH):
                        if is_q and n4 * 512 >= pair_budget:
                            continue
                        pt = ps.tile([P, 1024], f32, name="ps")[:, :512]
                        for d in range(8):
                            nc.tensor.matmul(
                                pt,
                                wt[d][:, mt * P:(mt + 1) * P],
                                x_sb[d][:, n4 * 512:(n4 + 1) * 512],
                                start=(d == 0), stop=(d == 7),
                            )
                        dslice = dst[mt][:, n4 * 512:(n4 + 1) * 512]
                        if is_q:
                            # fold the row mask into q (masked query -> q=0)
                            nc.vector.tensor_mul(
                                dslice, pt,
                                mk_sb[mt][:, n4 * 512:(n4 + 1) * 512])
                        else:
                            nc.vector.tensor_copy(dslice, pt)

            # v projection in (token, head-dim) layout + ones column
            for t in range(16):
                pt = ps.tile([P, 1024], f32, name="ps")[:, :GW]
                for d in range(8):
                    nc.tensor.matmul(
                        pt,
                        x_sb[d][:, t * P:(t + 1) * P],
                        wv_sb[d][:],
                        start=(d == 0), stop=(d == 7),
                    )
                nc.any.memset(v_sb[t][:], 1.0)  # ones column at [:, :, DH]
                nc.vector.tensor_copy(
                    v_sb[t][:, :, 0:DH],
                    pt.rearrange("p (h e) -> p h e", h=HPG),
                )

        # ---- phase 2 pools ----------------------------------------------
        rot = ctx.enter_context(tc.tile_pool(name="rot", bufs=4))
        ost = ctx.enter_context(tc.tile_pool(name="ost", bufs=3))
        sml = ctx.enter_context(tc.tile_pool(name="sml", bufs=3))
        scr = ctx.enter_context(tc.tile_pool(name="scr", bufs=3, space="DRAM"))

        # ---- mean of V per head (output of fully-masked query rows) -----
        if min(nchunks) < NCH:
            pmv = ps.tile([P, 1024], f32, name="ps")[:DH + 1, :HPG]
            for h in range(HPG):
                for jt in range(16):
                    nc.tensor.matmul(
                        pmv[:, h:h + 1],
                        v_sb[jt][:, h, :],
                        v_sb[jt][:, h, DH:DH + 1],  # the ones column
                        start=(jt == 0), stop=(jt == 15),
                    )
            nc.scalar.activation(meanv[:], pmv[:DH, :], COPY,
                                 bias=0.0, scale=1.0 / S)

        # ---- attention + output projection, per 512-query chunk ---------
        for i4 in range(NCH):
            qs = slice(i4 * 512, (i4 + 1) * 512)
            for pair in range(2):
                act_rr = [rr for rr in range(2)
                          if i4 < nchunks[2 * pair + rr]]
                pav = {rr: psav.tile([DH + 1, 512], f32, name="psav")
                       for rr in act_rr}
                if act_rr:
                    for jtp in range(8):
                        pse = {rr: ps.tile([P, 1024], f32, name="ps")
                               for rr in act_rr}
                        for jj in range(2):
                            jt = jtp * 2 + jj
                            for rr in act_rr:
                                rows = slice(64 * rr, 64 * rr + 64)
                                # scores^T = k @ q^T, head (2*pair + rr)
                                nc.tensor.matmul(
                                    pse[rr][:, jj * 512:(jj + 1) * 512],
                                    k_sb[pair][rows, jt * P:(jt + 1) * P],
                                    q_sb[pair][rows, qs],
                                    start=True, stop=True,
                                )
                        ex = {rr: rot.tile([P, 1024], bf16, name="ex")
                              for rr in act_rr}
                        for rr in act_rr:
                            nc.scalar.activation(
                                ex[rr][:], pse[rr][:], EXP,
                                bias=0.0, scale=0.125)
                        for rr in act_rr:
                            h = 2 * pair + rr
                            for jj in range(2):
                                jt = jtp * 2 + jj
                                nc.tensor.matmul(
                                    pav[rr][:],
                                    v_sb[jt][:, h, :],
                                    ex[rr][:, jj * 512:(jj + 1) * 512],
                                    start=(jtp == 0 and jj == 0),
                                    stop=(jtp == 7 and jj == 1),
                                )
                for rr in act_rr:
                    rc = sml.tile([1, 512], f32, name="rc")
                    nc.vector.reciprocal(rc[:], pav[rr][DH:DH + 1, :])
                    # partition-broadcast via a DRAM bounce (DMA cannot
                    # read SBUF with partition step 0; DRAM sources can)
                    sc = scr.tile([1, 512], f32, name="sc")
                    nc.sync.dma_start(sc[:], rc[:])
                    rcb = sml.tile([64, 512], f32, name="rcb")
                    nc.sync.dma_start(rcb[:], sc[:].partition_broadcast(64))
                    nc.vector.tensor_mul(
                        a_sb[pair][64 * rr:64 * rr + 64, qs],
                        pav[rr][0:DH, :],
                        rcb[:],
                    )
                for rr in range(2):
                    if rr not in act_rr:  # fully-masked chunk: mean-V fill
                        h = 2 * pair + rr
                        nc.vector.tensor_copy(
                            a_sb[pair][64 * rr:64 * rr + 64, qs],
                            meanv[:, h:h + 1].to_broadcast((64, 512)),
                        )
            # output projection for this query chunk
            for t4 in range(4):
                t = i4 * 4 + t4
                ot = ost.tile([P, D], f32, name="ot")
                for n2 in range(2):
                    pf = ps.tile([P, 1024], f32, name="ps")[:, :512]
                    for c in range(2):
                        nc.tensor.matmul(
                            pf,
                            a_sb[c][:, t * P:(t + 1) * P],
                            wo_sb[c][:, n2 * 512:(n2 + 1) * 512],
                            start=(c == 0), stop=(c == 1),
                        )
                    nc.vector.tensor_copy(ot[:, n2 * 512:(n2 + 1) * 512], pf)
                nc.sync.dma_start(out[t * P:(t + 1) * P, :], ot[:])


def build_program(budgets):
    """Build + schedule + compile the per-core program (cached per key)."""
    budgets = tuple(budgets)
    if budgets in _PROGS:
        return _PROGS[budgets]

    import concourse.mybir as mybir
    import concourse.tile as tile
    from concourse import bacc

    nc = bacc.Bacc("TRN2", target_bir_lowering=False, debug=False)
    f32 = mybir.dt.float32
    bf16 = mybir.dt.bfloat16
    aps = {
        "xT": nc.dram_tensor("xT", [D, S], bf16, kind="ExternalInput").ap(),
        "wqT": nc.dram_tensor("wqT", [D, GW], bf16, kind="ExternalInput").ap(),
        "wkT": nc.dram_tensor("wkT", [D, GW], bf16, kind="ExternalInput").ap(),
        "wvT": nc.dram_tensor("wvT", [D, GW], bf16, kind="ExternalInput").ap(),
        "woT": nc.dram_tensor("woT", [GW, D], bf16, kind="ExternalInput").ap(),
        "mask": nc.dram_tensor("mask", [2, P, S], f32,
                               kind="ExternalInput").ap(),
        "out": nc.dram_tensor("out", [S, D], f32, kind="ExternalOutput").ap(),
    }
    with tile.TileContext(nc) as tc:
        _emit(tc, aps, budgets)
    nc.compile()
    _PROGS[budgets] = nc
    return nc


def plan(valid_lens):
    """Head->core assignment and the compile-time budget tuple.

    Returns (budgets, heads_per_core): heads_per_core[c] lists the 4
    global head indices (within core c's batch) in slot order.
    """
    valid = np.asarray(valid_lens).reshape(B, H)
    heads_per_core = [None] * N_CORES
    quart_max = [0] * HPG
    for b in range(B):
        order = np.argsort(-valid[b], kind="stable")
        for j in range(HPG):
            hs = [int(order[4 * i + j]) for i in range(HPG)]
            heads_per_core[b * HPG + j] = hs
        for i in range(HPG):
            quart_max[i] = max(quart_max[i],
                               int(valid[b, order[4 * i]]))
    budgets = tuple(min(-(-m // 512) * 512, S) for m in quart_max)
    return budgets, heads_per_core


def make_in_maps(X, Wq, Wk, Wv, Wo, valid_lens):
    """Host-side sharding: build the 8 per-core input maps."""
    X = np.asarray(X, dtype=np.float32)
    valid = np.asarray(valid_lens).reshape(B, H)
    budgets, heads_per_core = plan(valid_lens)
    iota = np.arange(S)
    in_maps = []
    xTs = [_to_bf16(X[b].T) for b in range(B)]
    Wq, Wk, Wv, Wo = (np.asarray(a, np.float32) for a in (Wq, Wk, Wv, Wo))
    for c in range(N_CORES):
        b = c // HPG
        hs = heads_per_core[c]
        rows = np.concatenate([np.arange(h * DH, (h + 1) * DH) for h in hs])
        mask = np.empty((2, P, S), dtype=np.float32)
        for p in range(2):
            for rr in range(2):
                h = hs[2 * p + rr]
                mask[p, 64 * rr:64 * rr + 64, :] = (
                    iota < int(valid[b, h])).astype(np.float32)[None, :]
        in_maps.append({
            "xT": xTs[b],
            "wqT": _to_bf16(Wq[rows, :].T),
            "wkT": _to_bf16(Wk[rows, :].T),
            "wvT": _to_bf16(Wv[rows, :].T),
            "woT": _to_bf16(Wo[:, rows].T),
            "mask": mask,
        })
    return budgets, in_maps


def assemble(results, Wo, bv, bo):
    """Host-side unshard: sum row-parallel partials, fold bv/bo exactly."""
    out = np.zeros((B, S, D), dtype=np.float32)
    for c in range(N_CORES):
        b = c // HPG
        out[b] += results[c]["out"]
    bias = (np.asarray(bv, np.float32) @ np.asarray(Wo, np.float32).T
            + np.asarray(bo, np.float32))
    out += bias[None, None, :]
    return out


def _numpy_fallback(X, Wq, bq, Wk, bk, Wv, bv, Wo, bo, valid_lens):
    X = np.asarray(X, np.float32)
    q = (X @ np.asarray(Wq, np.float32).T + np.asarray(bq, np.float32))
    k = (X @ np.asarray(Wk, np.float32).T + np.asarray(bk, np.float32))
    v = (X @ np.asarray(Wv, np.float32).T + np.asarray(bv, np.float32))

    def split(y):
        return (y.reshape(B, S, H, DH).transpose(0, 2, 1, 3)
                .reshape(B * H, S, DH))

    q, k, v = split(q), split(k), split(v)
    s = np.einsum("bqd,bkd->bqk", q, k) / np.sqrt(DH).astype(np.float32)
    rm = (np.arange(S)[None, :]
          < np.asarray(valid_lens).reshape(-1)[:, None])
    s = np.where(rm[:, :, None], s, -1e6)
    s = s - s.max(axis=-1, keepdims=True)
    e = np.exp(s)
    attn = e / e.sum(axis=-1, keepdims=True)
    o = np.einsum("bqk,bkd->bqd", attn, v)
    o = o.reshape(B, H, S, DH).transpose(0, 2, 1, 3).reshape(B, S, D)
    return o @ np.asarray(Wo, np.float32).T + np.asarray(bo, np.float32)


def run_cores(budgets, in_maps, trace=False, **kw):
    """Run the compiled program on cores 0-7."""
    from concourse.bass_utils import run_bass_kernel_spmd

    nc = build_program(budgets)
    return run_bass_kernel_spmd(nc, in_maps, list(range(N_CORES)),
                                trace=trace, **kw)


def kernel(X, Wq, bq, Wk, bk, Wv, bv, Wo, bo, valid_lens):
    if np.any(np.asarray(bq)) or np.any(np.asarray(bk)):
        # never the case for this problem's setup_inputs (zeros);
        # exact fallback kept for safety.
        return _numpy_fallback(X, Wq, bq, Wk, bk, Wv, bv, Wo, bo, valid_lens)
    budgets, in_maps = make_in_maps(X, Wq, Wk, Wv, Wo, valid_lens)
    res = run_cores(budgets, in_maps, trace=False)
    return assemble(res.results, Wo, bv, bo)


# revision 12
# speedup vs baseline: 1.5953x; 1.3106x over previous
"""Trainium2 Bass kernel: MultiHeadSelfAttention (B=2, S=2048, D=1024, H=16).

Self-contained. Accepts FULL inputs, returns FULL output.

Sharding (8 cores, SPMD, no collectives):
  core c -> batch b = c // 4, lane j = c % 4. Within a batch the 16 heads
  are sorted by valid_len (desc) and dealt round-robin to the 4 lanes, so
  slot i on every core holds a head from rank-quartet i. Each core computes
  q/k/v projections for its 4 heads, attention, and the row-parallel
  partial of the output projection (A @ Wo[:, heads].T, shape (S, D)).
  Host sums the 4 partials per batch.

The program is specialized to per-slot QUERY BUDGETS: budget[i] =
ceil(max valid_len in rank-quartet i / 512) * 512. Query chunks beyond a
slot's budget are entirely masked rows, whose attention output is exactly
uniform (= mean of V), so they are filled from a precomputed mean-V column
instead of being computed. One program serves all 8 cores; distinct
budget tuples (at most 35) compile separately and are cached.

Device-side math notes:
  - All matmuls run in bf16 (fp32 PSUM accumulation).
  - The reference masks ENTIRE query rows j >= valid_len to -1e6 before
    softmax, making those rows' attention exactly uniform (1/S each). For
    masked rows inside a computed chunk we multiply q by the row mask:
    masked query -> scores all 0 -> exp all 1 -> uniform attention.
  - No max-subtraction in softmax: scores/8 are bounded (|s| < ~10), so
    exp() cannot overflow in fp32 and softmax is scale-invariant anyway.
  - Softmax denominator comes free from a ones-column appended to V
    (attn@V_aug yields sum(exp) in the extra output row).
  - bq/bk/bv are zeros in this problem's setup_inputs. bv/bo are folded in
    EXACTLY on the host (rows of attn sum to 1, so attn@(v+bv) = attn@v+bv).
    If bq/bk were ever nonzero we fall back to a numpy reference path.
"""

import numpy as np

B, S, D = 2, 2048, 1024
H, DH = 16, 64
HPG = 4                 # heads per core
GW = HPG * DH           # 256
P = 128
N_CORES = 8
NCH = S // 512          # query chunks

_PROGS = {}             # budgets tuple -> compiled Bacc


def _to_bf16(a):
    import ml_dtypes
    return np.ascontiguousarray(np.asarray(a, dtype=np.float32)
                                .astype(ml_dtypes.bfloat16))


def _emit(tc, aps, budgets):
    """Emit the per-core program. budgets: 4 per-slot query budgets."""
    from contextlib import ExitStack

    import concourse.mybir as mybir

    nc = tc.nc
    f32 = mybir.dt.float32
    bf16 = mybir.dt.bfloat16
    EXP = mybir.ActivationFunctionType.Exp
    COPY = mybir.ActivationFunctionType.Copy

    xT, wqT, wkT, wvT, woT, mask, out = (
        aps["xT"], aps["wqT"], aps["wkT"], aps["wvT"], aps["woT"],
        aps["mask"], aps["out"],
    )
    nchunks = [b // 512 for b in budgets]     # computed chunks per slot
    qmax = max(budgets)

    ctx = ExitStack()
    with ctx:
        sb = ctx.enter_context(tc.tile_pool(name="sb", bufs=1))
        ps = ctx.enter_context(tc.tile_pool(name="ps", bufs=3, space="PSUM"))
        psav = ctx.enter_context(tc.tile_pool(name="psav", bufs=2, space="PSUM"))

        # persistent intermediates
        wo_sb = [sb.tile([P, D], bf16, name=f"wo{c}") for c in range(2)]
        for c in range(2):
            nc.sync.dma_start(wo_sb[c][:], woT[c * P:(c + 1) * P, :])
        q_sb = [sb.tile([P, S], bf16, name=f"q{p}") for p in range(2)]
        k_sb = [sb.tile([P, S], bf16, name=f"k{p}") for p in range(2)]
        v_sb = [sb.tile([P, HPG, DH + 1], bf16, name=f"v{t}") for t in range(16)]
        a_sb = [sb.tile([P, S], bf16, name=f"a{c}") for c in range(2)]
        meanv = sb.tile([64, HPG], bf16, name="meanv")

        # ---- phase 1: projections (X/W/mask pool scoped to this phase) --
        with tc.tile_pool(name="xw", bufs=1) as xw:
            x_sb = [xw.tile([P, S], bf16, name=f"x{d}") for d in range(8)]
            for d in range(8):
                nc.sync.dma_start(x_sb[d][:], xT[d * P:(d + 1) * P, :])
            wq_sb = [xw.tile([P, GW], bf16, name=f"wq{d}") for d in range(8)]
            wk_sb = [xw.tile([P, GW], bf16, name=f"wk{d}") for d in range(8)]
            wv_sb = [xw.tile([P, GW], bf16, name=f"wv{d}") for d in range(8)]
            for d in range(8):
                nc.sync.dma_start(wq_sb[d][:], wqT[d * P:(d + 1) * P, :])
                nc.sync.dma_start(wk_sb[d][:], wkT[d * P:(d + 1) * P, :])
                nc.sync.dma_start(wv_sb[d][:], wvT[d * P:(d + 1) * P, :])
            mk_sb = [xw.tile([P, S], f32, name=f"mk{p}") for p in range(2)]
            for p in range(2):
                nc.sync.dma_start(mk_sb[p][:], mask[p])

            # q/k projections in (head-dim, token) layout; q-chunks beyond
            # the pair's budget are never read, so skip those matmuls
            for wt, dst, is_q in ((wq_sb, q_sb, True), (wk_sb, k_sb, False)):
                for mt in range(2):
                    pair_budget = max(budgets[2 * mt], budgets[2 * mt + 1])
                    for n4 in range(NCH):
                        if is_q and n4 * 512 >= pair_budget:
                            continue
                        pt = ps.tile([P, 1024], f32, name="ps")[:, :512]
                        for d in range(8):
                            nc.tensor.matmul(
                                pt,
                                wt[d][:, mt * P:(mt + 1) * P],
                                x_sb[d][:, n4 * 512:(n4 + 1) * 512],
                                start=(d == 0), stop=(d == 7),
                            )
                        dslice = dst[mt][:, n4 * 512:(n4 + 1) * 512]
                        if is_q:
                            # fold the row mask into q (masked query -> q=0)
                            nc.vector.tensor_mul(
                                dslice, pt,
                                mk_sb[mt][:, n4 * 512:(n4 + 1) * 512])
                        else:
                            nc.vector.tensor_copy(dslice, pt)

            # v projection in (token, head-dim) layout + ones column
            for t in range(16):
                pt = ps.tile([P, 1024], f32, name="ps")[:, :GW]
                for d in range(8):
                    nc.tensor.matmul(
                        pt,
                        x_sb[d][:, t * P:(t + 1) * P],
                        wv_sb[d][:],
                        start=(d == 0), stop=(d == 7),
                    )
                nc.any.memset(v_sb[t][:], 1.0)  # ones column at [:, :, DH]
                nc.vector.tensor_copy(
                    v_sb[t][:, :, 0:DH],
                    pt.rearrange("p (h e) -> p h e", h=HPG),
                )

        # ---- phase 2 pools ----------------------------------------------
        rot = ctx.enter_context(tc.tile_pool(name="rot", bufs=4))
        ost = ctx.enter_context(tc.tile_pool(name="ost", bufs=3))
        sml = ctx.enter_context(tc.tile_pool(name="sml", bufs=3))
        scr = ctx.enter_context(tc.tile_pool(name="scr", bufs=3, space="DRAM"))

        # ---- mean of V per head (output of fully-masked query rows) -----
        if min(nchunks) < NCH:
            pmv = ps.tile([P, 1024], f32, name="ps")[:DH + 1, :HPG]
            for h in range(HPG):
                for jt in range(16):
                    nc.tensor.matmul(
                        pmv[:, h:h + 1],
                        v_sb[jt][:, h, :],
                        v_sb[jt][:, h, DH:DH + 1],  # the ones column
                        start=(jt == 0), stop=(jt == 15),
                    )
            nc.scalar.activation(meanv[:], pmv[:DH, :], COPY,
                                 bias=0.0, scale=1.0 / S)

        # ---- attention + output projection, per 512-query chunk ---------
        def emit_final(i4):
            """Output projection for query chunk i4."""
            for t4 in range(4):
                t = i4 * 4 + t4
                ot = ost.tile([P, D], f32, name="ot")
                for n2 in range(2):
                    pf = ps.tile([P, 1024], f32, name="ps")[:, :512]
                    for c in range(2):
                        nc.tensor.matmul(
                            pf,
                            a_sb[c][:, t * P:(t + 1) * P],
                            wo_sb[c][:, n2 * 512:(n2 + 1) * 512],
                            start=(c == 0), stop=(c == 1),
                        )
                    nc.vector.tensor_copy(ot[:, n2 * 512:(n2 + 1) * 512], pf)
                nc.sync.dma_start(out[t * P:(t + 1) * P, :], ot[:])

        for i4 in range(NCH):
            qs = slice(i4 * 512, (i4 + 1) * 512)
            for pair in range(2):
                act_rr = [rr for rr in range(2)
                          if i4 < nchunks[2 * pair + rr]]
                for rr in act_rr:
                    h = 2 * pair + rr
                    rows = slice(64 * rr, 64 * rr + 64)
                    pav = psav.tile([DH + 1, 512], f32, name="psav")
                    # software pipeline: scores run one step ahead of
                    # exp+attn@V so the PE always has ready work
                    pses = []

                    def emit_scores(jtp):
                        pse = ps.tile([P, 1024], f32, name="ps")
                        for jj in range(2):
                            jt = jtp * 2 + jj
                            # scores^T = k @ q^T for head h
                            nc.tensor.matmul(
                                pse[:, jj * 512:(jj + 1) * 512],
                                k_sb[pair][rows, jt * P:(jt + 1) * P],
                                q_sb[pair][rows, qs],
                                start=True, stop=True,
                            )
                        pses.append(pse)

                    def emit_exp_av(jtp):
                        ex = rot.tile([P, 1024], bf16, name="ex")
                        nc.scalar.activation(ex[:], pses[jtp][:], EXP,
                                             bias=0.0, scale=0.125)
                        for jj in range(2):
                            jt = jtp * 2 + jj
                            nc.tensor.matmul(
                                pav[:],
                                v_sb[jt][:, h, :],
                                ex[:, jj * 512:(jj + 1) * 512],
                                start=(jtp == 0 and jj == 0),
                                stop=(jtp == 7 and jj == 1),
                            )

                    emit_scores(0)
                    for jtp in range(1, 8):
                        emit_scores(jtp)
                        emit_exp_av(jtp - 1)
                    emit_exp_av(7)

                    # softmax denominator -> reciprocal on 64 lanes via a
                    # DRAM re-partition bounce (DMA cannot read SBUF with
                    # partition step 0, and a 1-lane reciprocal is 3.3us)
                    rc = sml.tile([1, 512], f32, name="rc")
                    nc.vector.tensor_copy(rc[:], pav[DH:DH + 1, :])
                    sc = scr.tile([1, 512], f32, name="sc")
                    nc.sync.dma_start(sc[:], rc[:])
                    rs = sml.tile([64, 8], f32, name="rs")
                    nc.sync.dma_start(
                        rs[:], sc[:].rearrange("o (p j) -> (o p) j", p=64))
                    rr_t = sml.tile([64, 8], f32, name="rr")
                    nc.vector.reciprocal(rr_t[:], rs[:])
                    sc2 = scr.tile([1, 512], f32, name="sc2")
                    nc.sync.dma_start(
                        sc2[:].rearrange("o (p j) -> (o p) j", p=64), rr_t[:])
                    rcb = sml.tile([64, 512], f32, name="rcb")
                    nc.sync.dma_start(rcb[:], sc2[:].partition_broadcast(64))
                    nc.vector.tensor_mul(
                        a_sb[pair][rows, qs], pav[0:DH, :], rcb[:])
                for rr in range(2):
                    if rr not in act_rr:  # fully-masked chunk: mean-V fill
                        h = 2 * pair + rr
                        nc.vector.tensor_copy(
                            a_sb[pair][64 * rr:64 * rr + 64, qs],
                            meanv[:, h:h + 1].to_broadcast((64, 512)),
                        )
            # emit the PREVIOUS chunk's output projection here so its
            # matmuls fill the pipeline-drain gap at the chunk boundary
            if i4 > 0:
                emit_final(i4 - 1)
        emit_final(NCH - 1)


def build_program(budgets):
    """Build + schedule + compile the per-core program (cached per key)."""
    budgets = tuple(budgets)
    if budgets in _PROGS:
        return _PROGS[budgets]

    import concourse.mybir as mybir
    import concourse.tile as tile
    from concourse import bacc

    nc = bacc.Bacc("TRN2", target_bir_lowering=False, debug=False)
    f32 = mybir.dt.float32
    bf16 = mybir.dt.bfloat16
    aps = {
        "xT": nc.dram_tensor("xT", [D, S], bf16, kind="ExternalInput").ap(),
        "wqT": nc.dram_tensor("wqT", [D, GW], bf16, kind="ExternalInput").ap(),
        "wkT": nc.dram_tensor("wkT", [D, GW], bf16, kind="ExternalInput").ap(),
        "wvT": nc.dram_tensor("wvT", [D, GW], bf16, kind="ExternalInput").ap(),
        "woT": nc.dram_tensor("woT", [GW, D], bf16, kind="ExternalInput").ap(),
        "mask": nc.dram_tensor("mask", [2, P, S], f32,
                               kind="ExternalInput").ap(),
        "out": nc.dram_tensor("out", [S, D], f32, kind="ExternalOutput").ap(),
    }
    with tile.TileContext(nc) as tc:
        _emit(tc, aps, budgets)
    nc.compile()
    _PROGS[budgets] = nc
    return nc


def plan(valid_lens):
    """Head->core assignment and the compile-time budget tuple.

    Returns (budgets, heads_per_core): heads_per_core[c] lists the 4
    global head indices (within core c's batch) in slot order.
    """
    valid = np.asarray(valid_lens).reshape(B, H)
    heads_per_core = [None] * N_CORES
    quart_max = [0] * HPG
    for b in range(B):
        order = np.argsort(-valid[b], kind="stable")
        for j in range(HPG):
            hs = [int(order[4 * i + j]) for i in range(HPG)]
            heads_per_core[b * HPG + j] = hs
        for i in range(HPG):
            quart_max[i] = max(quart_max[i],
                               int(valid[b, order[4 * i]]))
    budgets = tuple(min(-(-m // 512) * 512, S) for m in quart_max)
    return budgets, heads_per_core


def make_in_maps(X, Wq, Wk, Wv, Wo, valid_lens):
    """Host-side sharding: build the 8 per-core input maps."""
    X = np.asarray(X, dtype=np.float32)
    valid = np.asarray(valid_lens).reshape(B, H)
    budgets, heads_per_core = plan(valid_lens)
    iota = np.arange(S)
    in_maps = []
    xTs = [_to_bf16(X[b].T) for b in range(B)]
    Wq, Wk, Wv, Wo = (np.asarray(a, np.float32) for a in (Wq, Wk, Wv, Wo))
    for c in range(N_CORES):
        b = c // HPG
        hs = heads_per_core[c]
        rows = np.concatenate([np.arange(h * DH, (h + 1) * DH) for h in hs])
        mask = np.empty((2, P, S), dtype=np.float32)
        for p in range(2):
            for rr in range(2):
                h = hs[2 * p + rr]
                mask[p, 64 * rr:64 * rr + 64, :] = (
                    iota < int(valid[b, h])).astype(np.float32)[None, :]
        in_maps.append({
            "xT": xTs[b],
            "wqT": _to_bf16(Wq[rows, :].T),
            "wkT": _to_bf16(Wk[rows, :].T),
            "wvT": _to_bf16(Wv[rows, :].T),
            "woT": _to_bf16(Wo[:, rows].T),
            "mask": mask,
        })
    return budgets, in_maps


def assemble(results, Wo, bv, bo):
    """Host-side unshard: sum row-parallel partials, fold bv/bo exactly."""
    out = np.zeros((B, S, D), dtype=np.float32)
    for c in range(N_CORES):
        b = c // HPG
        out[b] += results[c]["out"]
    bias = (np.asarray(bv, np.float32) @ np.asarray(Wo, np.float32).T
            + np.asarray(bo, np.float32))
    out += bias[None, None, :]
    return out


def _numpy_fallback(X, Wq, bq, Wk, bk, Wv, bv, Wo, bo, valid_lens):
    X = np.asarray(X, np.float32)
    q = (X @ np.asarray(Wq, np.float32).T + np.asarray(bq, np.float32))
    k = (X @ np.asarray(Wk, np.float32).T + np.asarray(bk, np.float32))
    v = (X @ np.asarray(Wv, np.float32).T + np.asarray(bv, np.float32))

    def split(y):
        return (y.reshape(B, S, H, DH).transpose(0, 2, 1, 3)
                .reshape(B * H, S, DH))

    q, k, v = split(q), split(k), split(v)
    s = np.einsum("bqd,bkd->bqk", q, k) / np.sqrt(DH).astype(np.float32)
    rm = (np.arange(S)[None, :]
          < np.asarray(valid_lens).reshape(-1)[:, None])
    s = np.where(rm[:, :, None], s, -1e6)
    s = s - s.max(axis=-1, keepdims=True)
    e = np.exp(s)
    attn = e / e.sum(axis=-1, keepdims=True)
    o = np.einsum("bqk,bkd->bqd", attn, v)
    o = o.reshape(B, H, S, DH).transpose(0, 2, 1, 3).reshape(B, S, D)
    return o @ np.asarray(Wo, np.float32).T + np.asarray(bo, np.float32)


def run_cores(budgets, in_maps, trace=False, **kw):
    """Run the compiled program on cores 0-7."""
    from concourse.bass_utils import run_bass_kernel_spmd

    nc = build_program(budgets)
    return run_bass_kernel_spmd(nc, in_maps, list(range(N_CORES)),
                                trace=trace, **kw)


def kernel(X, Wq, bq, Wk, bk, Wv, bv, Wo, bo, valid_lens):
    if np.any(np.asarray(bq)) or np.any(np.asarray(bk)):
        # never the case for this problem's setup_inputs (zeros);
        # exact fallback kept for safety.
        return _numpy_fallback(X, Wq, bq, Wk, bk, Wv, bv, Wo, bo, valid_lens)
    budgets, in_maps = make_in_maps(X, Wq, Wk, Wv, Wo, valid_lens)
    res = run_cores(budgets, in_maps, trace=False)
    return assemble(res.results, Wo, bv, bo)
